# revision 18
# baseline (speedup 1.0000x reference)
"""Trainium2 Bass kernel for nn_DeepFilter — pipelined multi-body design.

Math: out_r = Box_{3x5}(xr*fr - xi*fi), out_i = 2*Box(xr*fi) where Box is a
(2I+1)x(2L+1)=3x5 box filter over (freq d, time t) with zero padding.

Device-side restructure (per core, pure data parallelism over B):
  - Host pre-transposes inputs to [T, D] f16, pre-negates xi, and applies the
    imag 2x on the way out, so the device does only:
      products (DVE, f16 2x mode, 4 fused full-width ops):
        Pr = xr*fr + (-xi)*fi,  Pi = xr*fi
      one PE stage: psum[t_out, d] = sum_kf sum_{t_in} b2[t_in, t_out] *
        P[t_in, d+kf]   (stationary = constant time-box band b2, moving =
        freq-shifted slices of P; PSUM accumulates the 3 freq taps)
      ACT: psum->SBUF f16 copies
      DMA out, host reorders back.
  - t is tiled into 17 overlapped chunks of 128 rows (stride 124 = 128-2L) so
    each PE window needs exactly one chunk; d is padded by I=1 zero col each
    side inside the chunk (width 258) so freq taps are plain column shifts.

Performance structure (vs the serial one-body design):
  - Chunk-major SBUF input layout [P, NW, 4*CW]: each chunk DMA is one
    contiguous 2064B run per partition (vs 4x516B) -> near line-rate SDMA.
  - ALL bulk DMA on the sync (SP) HWDGE ring: it fans across 16 SDMA
    engines; the scalar ring only gets 4 and was the baseline bottleneck.
  - K bodies unrolled inside the hardware rep-loop with explicit A/B
    double-buffered XT/P2/t2, so body j+1's input DMAs stream while body j
    computes; the ~2us back-edge barrier is amortized over K evaluations.
  - Emission order interleaves body j+1 input DMAs BEFORE body j output
    DMAs on the SP queue so output-copy waits don't head-of-line-block the
    input stream.
  - Output DRAM layout [W, NW, 512] matches the SBUF tiles: contiguous
    2048B per partition per pair store; host un-permutes (free).
"""

import os
import sys

os.environ.setdefault("BASS_NEVER_TRACE", "1")

if "/opt/trn_rl_repo" not in sys.path:
    sys.path.insert(0, "/opt/trn_rl_repo")

import numpy as np

_CACHE = {}
LAST_RESULTS = None
N_CORES = 8

P = 128
K_BODIES = 5  # evaluations per hardware-loop trip (reps % K_BODIES == 0)


def _install_drain_patch():
    """walrus in this env rejects instructions with >2 sem waits; Tile's tail
    drain carries one wait per live proc.  Split them across SP no-ops."""
    import bass_rust
    from concourse import tile as _tile

    if getattr(_tile.TileContext, "_drain_patch_installed", False):
        return

    def _split_drain_and_barrier(self, tick_clock, wait_clock):
        nc = self.nc
        g = tick_clock.global_clock
        vals = list(g)
        n = len(vals)
        for i, v in enumerate(vals):
            if v <= 0:
                continue
            part = bass_rust.VectorClock([v if j == i else 0 for j in range(n)])
            nop = nc.sync.nop(nofuse=True)
            wait_clock.add_sem_waits(nop.ins, bass_rust.ScopedClock({None: part}))
        nc.sync.drain()
        nc.all_engine_barrier()
        assert self.sems is not None
        popped = nc._tile_sem_poison_stack.pop()
        assert popped is self._sem_poison
        nc.clear_and_free_semaphores(list(self.sems.allocated().values()))
        nc.all_engine_barrier()

    _tile.TileContext._drain_and_barrier = _split_drain_and_barrier
    _tile.TileContext._drain_patch_installed = True


_MAX_WAITS = 1


def _split_excess_waits(nc):
    """walrus codegen rejects instructions carrying more than ~2 sem waits.
    Move excess waits onto same-engine no-ops placed just before the
    instruction."""
    from concourse import mybir

    uid = 0
    for fn in nc.m.functions:
        for bb in fn.blocks:
            insts = bb.instructions
            out = []
            changed = False
            for inst in insts:
                si = inst.sync_info
                waits = list(si.on_wait) if si is not None else []
                if len(waits) > _MAX_WAITS:
                    changed = True
                    extra, keep = waits[:-_MAX_WAITS], waits[-_MAX_WAITS:]
                    for i in range(0, len(extra), _MAX_WAITS):
                        chunk = extra[i : i + _MAX_WAITS]
                        nop = mybir.InstNoOp(name=f"wsplit-{uid}", ins=[], outs=[])
                        uid += 1
                        nop.engine = inst.engine
                        nop.sync_info = mybir.SyncInfo(on_wait=chunk, on_update=[])
                        out.append(nop)
                    inst.sync_info = mybir.SyncInfo(
                        on_wait=keep, on_update=list(si.on_update)
                    )
                out.append(inst)
            if changed:
                bb.instructions = out


def _build_program(D, T, L, I, reps=1):
    import contextlib

    import concourse.bass as bass
    import concourse.tile as tile
    from concourse import mybir

    _install_drain_patch()

    f32 = mybir.dt.float32
    f16 = mybir.dt.float16
    assert D == 256 and I == 1, (D, I)
    W = P - 2 * L                # complete outputs per window (124)
    NW = (T + W - 1) // W        # windows == overlapped t-chunks (17)
    CW = D + 2 * I               # padded chunk width (258)
    ROWE = 4 * CW                # packed row elems (1032)
    FLAT = NW * CW
    NPAIR = NW // 2              # full window pairs (8)
    LASTM = T - W * (NW - 1)     # outputs in last window (64)
    D2 = 2 * D                   # out row elems per window (512)

    K = 1 if reps == 1 else K_BODIES
    assert reps % K == 0, (reps, K)

    nc = bass.Bass()
    TPAD = W * (NW - 1) + P - T        # zero pad rows: L at top, rest at tail
    d_x = nc.dram_tensor("x", [T + TPAD, 4, CW], f16, kind="ExternalInput")
    # band padded to [P, P]: cols W..P-1 are zero, so full-pair matmuls
    # produce 128 output partitions (rows W..P-1 = 0) and stores can use
    # 128 partitions without any extra zero-fill work.
    d_b2 = nc.dram_tensor("band", [P, P], f16, kind="ExternalInput")
    # 128 rows (not 124): the HWDGE descriptor spray fans across
    # `largest divisor of partition-count <= 16` SDMA engines, so stores
    # must use a partition count divisible by 16 to reach all 16 engines.
    # Rows W..P-1 are zero filler the host drops.
    d_out = nc.dram_tensor("out", [P, NW, D2], f16, kind="ExternalOutput")
    d_cnt = (
        nc.dram_tensor("cnt", [P, 16], f32, kind="ExternalOutput")
        if reps > 1
        else None
    )

    with tile.TileContext(nc) as tc, contextlib.ExitStack() as stk:
        consts = stk.enter_context(tc.tile_pool(name="consts", bufs=1))
        xpool = stk.enter_context(tc.tile_pool(name="x", bufs=1))
        ppool = stk.enter_context(tc.tile_pool(name="p", bufs=1))
        pspool = stk.enter_context(tc.tile_pool(name="ps", bufs=3, space="PSUM"))
        pslast = stk.enter_context(tc.tile_pool(name="psl", bufs=1, space="PSUM"))
        opool = stk.enter_context(tc.tile_pool(name="o", bufs=8))
        olpool = stk.enter_context(tc.tile_pool(name="ol", bufs=2))

        cnt_t = None
        if d_cnt is not None:
            cnt_t = consts.tile([P, 16], f32, tag="cnt")
            nc.vector.memset(cnt_t, 0.0)
        b2 = consts.tile([P, P], f16, tag="b2")
        nc.sync.dma_start(out=b2, in_=d_b2[:, :])

        # Explicit double buffers (hardware-loop bodies have static
        # addresses, so rotation must be by body index, not pool order).
        NBUF = min(2, K)
        XT = [
            xpool.tile([P, NW, ROWE], f16, tag=f"XT{i}", name=f"XT{i}")
            for i in range(NBUF)
        ]
        P2 = [
            ppool.tile([P, 2, FLAT], f16, tag=f"P2{i}", name=f"P2{i}")
            for i in range(NBUF)
        ]
        T2 = [
            ppool.tile([P, FLAT], f16, tag=f"t2{i}", name=f"t2{i}")
            for i in range(NBUF)
        ]
        # Freq-pair sums: S[., 0, w*D+d] = Pr[w, d] + Pr[w, d+2] (likewise Pi),
        # so the PE needs only 2 accumulating taps per window (S at kf=0 and
        # P2 at kf=1) instead of 3 — 34 matmuls/body instead of 51.
        # The +2 shift keeps both DVE operands 4B-aligned (2x f16 mode).
        S2 = [
            ppool.tile([P, 2, NW * D], f16, tag=f"S2{i}", name=f"S2{i}")
            for i in range(NBUF)
        ]

        loop_stk = contextlib.ExitStack()
        if reps > 1:
            loop_stk.enter_context(tc.For_i(0, reps // K, 1))

        import bass_rust

        def _overlap_src_ap():
            """[P, NW, ROWE] view of d_x with overlapping t-chunks
            (chunk stride W=124 rows < chunk height P=128 rows)."""
            ap = d_x[0:P, :, :].rearrange("p c w -> p (c w)").copy()
            ap.ap = bass_rust.VecI64Pair(
                [(ROWE, P), (W * ROWE, NW), (1, ROWE)]
            )
            return ap

        def emit_in(j):
            xt = XT[j % NBUF]
            nc.sync.dma_start(out=xt[:, :, :], in_=_overlap_src_ap())

        def emit_products(j):
            xt, p2, t2, s2 = XT[j % NBUF], P2[j % NBUF], T2[j % NBUF], S2[j % NBUF]
            xr = xt[:, :, 0 * CW : 1 * CW]
            fr = xt[:, :, 1 * CW : 2 * CW]
            xin = xt[:, :, 2 * CW : 3 * CW]
            fi = xt[:, :, 3 * CW : 4 * CW]
            pr = p2[:, 0, :].rearrange("p (w c) -> p w c", c=CW)
            pi = p2[:, 1, :].rearrange("p (w c) -> p w c", c=CW)
            t2v = t2.rearrange("p (w c) -> p w c", c=CW)
            sr = s2[:, 0, :].rearrange("p (w c) -> p w c", c=D)
            si = s2[:, 1, :].rearrange("p (w c) -> p w c", c=D)
            nc.vector.tensor_mul(pr, xr, fr)
            nc.vector.tensor_mul(t2v, xin, fi)
            nc.vector.tensor_mul(pi, xr, fi)
            nc.vector.tensor_add(pr, pr, t2v)
            nc.vector.tensor_add(sr, pr[:, :, 0:D], pr[:, :, 2 : 2 + D])
            nc.vector.tensor_add(si, pi[:, :, 0:D], pi[:, :, 2 : 2 + D])
            if cnt_t is not None:
                nc.vector.tensor_scalar_add(cnt_t, cnt_t, 1.0)

        def emit_pairs(j):
            """matmuls (PE) + psum->sbuf copies (ACT); returns out tiles."""
            p2, s2 = P2[j % NBUF], S2[j % NBUF]
            outs = []
            for pair in range(NPAIR + 1):
                last = pair == NPAIR
                M = LASTM if last else P
                ncols = D2 if last else 2 * D2
                pool = pslast if last else pspool
                ps = pool.tile(
                    [M, ncols], f32, tag="psl" if last else "ps", name="ps"
                )
                for s in range(1 if last else 2):
                    w = 2 * pair + s
                    movA = s2[:, :, D * w : D * w + D]
                    movB = p2[:, :, CW * w + 1 : CW * w + 1 + D]
                    nc.tensor.matmul(
                        ps[0:M, s * D2 : (s + 1) * D2],
                        b2[:, 0:M], movA, start=True, stop=False,
                    )
                    nc.tensor.matmul(
                        ps[0:M, s * D2 : (s + 1) * D2],
                        b2[:, 0:M], movB, start=False, stop=True,
                    )
                opool_ = olpool if last else opool
                o = opool_.tile([M, ncols], f16, tag="ol" if last else "o", name="o")
                nc.scalar.copy(o[:, :], ps[:, :])
                outs.append(o)
            return outs

        def emit_out(j, outs):
            for pair in range(NPAIR):
                nc.sync.dma_start(
                    out=d_out[:, 2 * pair : 2 * pair + 2, :],
                    in_=outs[pair].rearrange("q (s d) -> q s d", s=2),
                )
            nc.sync.dma_start(out=d_out[0:LASTM, NW - 1, :], in_=outs[NPAIR])

        # Software-pipelined emission: body j+1's input DMAs are enqueued on
        # the SP ring BEFORE body j's output DMAs, so the (copy-gated) output
        # dispatches never head-of-line-block the input stream.
        emit_in(0)
        pending = None
        for j in range(K):
            emit_products(j)
            outs = emit_pairs(j)
            if j + 1 < K:
                emit_in(j + 1)
            if pending is not None:
                emit_out(j - 1, pending)
            pending = outs
        emit_out(K - 1, pending)

        loop_stk.close()  # exit For_i (pools stay open)
        if cnt_t is not None:
            nc.sync.dma_start(out=d_cnt[:, :], in_=cnt_t)

    _split_excess_waits(nc)
    return nc


def _get_program(D, T, L, I, reps=1):
    key = (D, T, L, I, reps)
    if key not in _CACHE:
        _CACHE[key] = _build_program(D, T, L, I, reps)
    return _CACHE[key]


def _band(T, L):
    W = P - 2 * L
    b2 = np.zeros((P, P), dtype=np.float16)  # cols W..P-1 stay zero
    for p in range(P):
        for n in range(W):
            if 0 <= p - n <= 2 * L:
                b2[p, n] = 1.0
    return b2


def _prep_inputs(inputs_r, inputs_i, filters_r, filters_i, L, I):
    B, D, T = inputs_r.shape
    band = _band(T, L)
    in_maps = []
    for b in range(B):
        W, NW = P - 2 * L, (T + P - 2 * L - 1) // (P - 2 * L)
        TPAD = W * (NW - 1) + P - T
        xcat = np.zeros((T + TPAD, 4, D + 2 * I), dtype=np.float16)
        xcat[L : L + T, 0, I : I + D] = inputs_r[b].T
        xcat[L : L + T, 1, I : I + D] = filters_r[b].T
        xcat[L : L + T, 2, I : I + D] = -inputs_i[b].T
        xcat[L : L + T, 3, I : I + D] = filters_i[b].T
        in_maps.append({"x": xcat, "band": band})
    return in_maps


_RUNNER_CACHE = {}


def _get_runner(nc, n_cores):
    """Persistent jitted executor for `nc` (avoids per-call retracing)."""
    key = (id(nc), n_cores)
    if key in _RUNNER_CACHE:
        return _RUNNER_CACHE[key]

    import jax
    from jax.experimental.shard_map import shard_map
    from jax.sharding import Mesh, PartitionSpec

    from concourse import bass2jax, mybir

    bass2jax.install_neuronx_cc_hook()
    partition_name = nc.partition_id_tensor.name if nc.partition_id_tensor else None
    in_names, out_names, out_avals, out_shapes = [], [], [], []
    for alloc in nc.m.functions[0].allocations:
        if not isinstance(alloc, mybir.MemoryLocationSet):
            continue
        name = alloc.memorylocations[0].name
        if alloc.kind == "ExternalInput":
            if name != partition_name:
                in_names.append(name)
        elif alloc.kind == "ExternalOutput":
            shape = tuple(alloc.tensor_shape)
            dtype = mybir.dt.np(alloc.dtype)
            out_names.append(name)
            out_avals.append(jax.core.ShapedArray(shape, dtype))
            out_shapes.append((shape, dtype))
    n_params = len(in_names)
    all_names = in_names + out_names
    if partition_name is not None:
        all_names.append(partition_name)
    donate = tuple(range(n_params, n_params + len(out_names)))

    def _body(*args):
        operands = list(args)
        if partition_name is not None:
            operands.append(bass2jax.partition_id_tensor())
        outs = bass2jax._bass_exec_p.bind(
            *operands,
            out_avals=tuple(out_avals),
            in_names=tuple(all_names),
            out_names=tuple(out_names),
            lowering_input_output_aliases=(),
            sim_require_finite=True,
            sim_require_nnan=True,
            nc=nc,
        )
        return tuple(outs)

    devices = jax.devices()[:n_cores]
    mesh = Mesh(np.asarray(devices), ("core",))
    in_specs = (PartitionSpec("core"),) * (n_params + len(out_names))
    out_specs = (PartitionSpec("core"),) * len(out_names)
    sharded = jax.jit(
        shard_map(
            _body, mesh=mesh, in_specs=in_specs, out_specs=out_specs,
            check_rep=False,
        ),
        donate_argnums=donate,
        keep_unused=True,
    )

    def run(in_maps):
        n = len(in_maps)
        assert n == n_cores
        concat_in = [
            np.concatenate([np.asarray(m[nm])[None] for m in in_maps], axis=0).reshape(
                n * np.asarray(in_maps[0][nm]).shape[0],
                *np.asarray(in_maps[0][nm]).shape[1:],
            )
            for nm in in_names
        ]
        zeros = [np.zeros((n * s[0], *s[1:]), dt) for (s, dt) in out_shapes]
        outs = sharded(*concat_in, *zeros)
        return [
            {
                nm: np.asarray(outs[i]).reshape(n, *out_shapes[i][0])[c]
                for i, nm in enumerate(out_names)
            }
            for c in range(n)
        ]

    _RUNNER_CACHE[key] = run
    return run


def kernel(inputs_r, inputs_i, filters_r, filters_i, L, I):
    global LAST_RESULTS
    from concourse.bass_utils import run_bass_kernel_spmd

    L = int(L)
    I = int(I)
    xr = np.asarray(inputs_r, dtype=np.float32)
    xi = np.asarray(inputs_i, dtype=np.float32)
    fr = np.asarray(filters_r, dtype=np.float32)
    fi = np.asarray(filters_i, dtype=np.float32)
    B, D, T = xr.shape
    W = P - 2 * L
    NW = (T + W - 1) // W

    nc = _get_program(D, T, L, I)
    in_maps_all = _prep_inputs(xr, xi, fr, fi, L, I)

    outs = []
    step = min(B, N_CORES)
    for s in range(0, B, step):
        batch = list(range(s, min(s + step, B)))
        in_maps = [in_maps_all[b] for b in batch]
        try:
            runner = _get_runner(nc, len(batch))
            results = runner(in_maps)
        except Exception:
            results = run_bass_kernel_spmd(
                nc, in_maps, core_ids=list(range(len(batch)))
            ).results
        LAST_RESULTS = results
        for i in range(len(batch)):
            ob = results[i]["out"].astype(np.float32)   # [P, NW, 2D]
            a = ob.transpose(1, 0, 2)[:, 0:W, :].reshape(NW * W, 2 * D)[0:T]
            full = np.empty((2 * D, T), dtype=np.float32)
            full[0:D] = a[:, 0:D].T
            full[D : 2 * D] = a[:, D : 2 * D].T * 2.0
            outs.append(full)
    return np.stack(outs, axis=0)
